# revision 1
# baseline (speedup 1.0000x reference)
"""Trainium2 Bass kernel for the additive-attention module.

Per-core computation (data-parallel over batch, 8 cores, 32 batches each):
  att_enc = enc @ W_enc            [6272, 2048] @ [2048, 512]  (dominant matmul)
  att_dec = dec @ W_dec + b_dec    [32, 512] @ [512, 512]
  hidden  = relu(att_enc + att_dec[b] + b_enc)   (computed transposed: [a, row])
  att     = hidden @ W_fin                        (b_fin dropped: softmax shift-invariant)
  w       = softmax_p(att)         (no max-subtraction: |att| < ~2)
  out     = sum_p w[b,p] * enc[b,p,:]

Layouts on chip (per 128-row chunk c of the flattened [6272, 2048] encoder):
  nat_c  [128 rows, 2048 e]  bf16   (gpsimd cast-DMA from f32 HBM)
  encT   [128 e,  rows]      bf16   (HWDGE DMA-transpose of nat tiles)
  step1: att_encT[a, rows] accumulated over 16 e-chunks in PSUM, bias+relu via
         ScalarE (per-partition bias = att_decT column) -> hidT bf16
  step4: attT column [128 rows, 1] = sum_aj hidT_aj.T @ W_fin_aj; exp on ScalarE
  step6: block-diagonal weights W6_c [128, 32] (softmax numerators scattered
         per batch column via precomputed int8 masks + copy_predicated) so
         out accumulates as W6.T @ nat_c over all 49 chunks. Chunk c targets
         PE column-strip 32*(c%4) (tile_position col packing -> 4 concurrent
         streams); strips and the rhs=ones Z column are reduced with a
         stacked-identity f32 matmul, then scaled by 1/Z on ScalarE.
"""

import sys

try:
    import concourse.bass as bass  # noqa: F401
except ImportError:
    sys.path.insert(0, "/opt/trn_rl_repo")

import numpy as np

import concourse.bass as bass
import concourse.mybir as mybir
import concourse.tile as tile
from concourse import bacc
from concourse.bass_utils import run_bass_kernel_spmd
from concourse.masks import make_identity

F32 = mybir.dt.float32
BF16 = mybir.dt.bfloat16
AF = mybir.ActivationFunctionType

N_CORES = 8
B = 256
B_LOC = B // N_CORES  # 32
P = 196
E = 2048
A = 512
W = 512
ROWS = B_LOC * P  # 6272
NCHUNK = (ROWS + 127) // 128  # 49
EJ = E // 128  # 16
AJ = A // 128  # 4
WJ = W // 128  # 4
EG = E // 512  # 4


def _batch_segments(r0, nrows):
    """Batch segments of global row range [r0, r0+nrows): (batch, local_s0, local_s1)."""
    segs = []
    b0 = r0 // P
    b1 = (r0 + nrows - 1) // P
    for b in range(b0, b1 + 1):
        s0 = max(b * P - r0, 0)
        s1 = min((b + 1) * P - r0, nrows)
        if s1 > s0:
            segs.append((b, s0, s1))
    return segs


def build():
    nc = bacc.Bacc()

    enc_x = nc.dram_tensor("encoder_out", [ROWS, E], F32, kind="ExternalInput")
    dec_x = nc.dram_tensor("decoder_out", [B_LOC, W], F32, kind="ExternalInput")
    wenc_x = nc.dram_tensor("W_enc", [E, A], F32, kind="ExternalInput")
    benc_x = nc.dram_tensor("b_enc", [1, A], F32, kind="ExternalInput")
    wdec_x = nc.dram_tensor("W_dec", [W, A], F32, kind="ExternalInput")
    bdec_x = nc.dram_tensor("b_dec", [1, A], F32, kind="ExternalInput")
    wfin_x = nc.dram_tensor("W_fin", [A], F32, kind="ExternalInput")
    out_x = nc.dram_tensor("out", [B_LOC, E], F32, kind="ExternalOutput")

    with tile.TileContext(nc) as tc:
        with tc.tile_pool(name="consts", bufs=1) as consts:
            # replicated weights, bf16, e/w-chunk-major blocks of 512 columns.
            # NOTE: this pool stays open for the whole kernel (everything below
            # is nested inside) — its tiles are used throughout.
            identity = consts.tile([128, 128], F32)
            make_identity(nc, identity[:])
            wfin_sb = consts.tile([128, AJ], BF16)
            nc.gpsimd.dma_start(wfin_sb[:], wfin_x.rearrange("(j p) -> p j", p=128))
            # W_enc first half now; second half goes into the Pool ring after
            # group 0's loads (see main loop) to shorten the startup ramp
            w_enc_sb = consts.tile([128, EJ * A], BF16)
            nc.gpsimd.dma_start(
                w_enc_sb[:, 0 : EJ // 2 * A],
                wenc_x[0 : EJ // 2 * 128, :].rearrange("(j p) a -> p j a", p=128),
            )
            w_dec_sb = consts.tile([128, WJ * A], BF16)
            nc.gpsimd.dma_start(w_dec_sb[:], wdec_x.rearrange("(j p) a -> p j a", p=128))
            ones32 = consts.tile([1, 32], BF16)
            nc.vector.memset(ones32[:], 1.0)
            onescol = consts.tile([128, 1], BF16)
            nc.vector.memset(onescol[:], 1.0)

            dec_sb = consts.tile([B_LOC, W], F32)
            nc.sync.dma_start(dec_sb[:], dec_x[:])
            benc_sb = consts.tile([1, A], F32)
            nc.sync.dma_start(benc_sb[:], benc_x[:])
            bdec_sb = consts.tile([1, A], F32)
            nc.sync.dma_start(bdec_sb[:], bdec_x[:])
            bb_f = consts.tile([1, A], F32)
            nc.vector.tensor_add(bb_f[:], benc_sb[:], bdec_sb[:])
            bb_bf = consts.tile([1, A], BF16)
            nc.vector.tensor_copy(bb_bf[:], bb_f[:])

            decT_bf = consts.tile([128, WJ * B_LOC], BF16)
            biasT_sb = consts.tile([128, AJ * B_LOC], F32)
            attT_sb = consts.tile([128, NCHUNK], BF16)
            out_sb = consts.tile([B_LOC, E], F32)
            recip_z = consts.tile([B_LOC, 1], F32)

            # per-chunk batch-membership masks: mask_c[p, b] = 1 iff row
            # 128c+p belongs to batch b, i.e. 0 <= 128c + p - 196b <= 195.
            # iota = channel_multiplier*p + pattern_step*b + base; TRUE keeps
            # in_, FALSE writes fill.
            id4 = consts.tile([128, B_LOC], F32)
            nc.gpsimd.memset(id4[:], 0.0)
            for k in range(4):
                nc.gpsimd.affine_select(
                    id4[:], id4[:], pattern=[[-1, B_LOC]],
                    compare_op=mybir.AluOpType.not_equal, fill=1.0,
                    base=-B_LOC * k, channel_multiplier=1,
                )
            ones_pb = consts.tile([128, B_LOC], mybir.dt.int8)
            nc.vector.memset(ones_pb[:], 1)
            masks_sb = consts.tile([128, NCHUNK * B_LOC], mybir.dt.int8)

            def issue_mask(c):
                # mask built lazily right before its chunk's step6 so the 98
                # Pool-engine ops spread across the ring instead of clogging
                # the startup
                m = masks_sb[:, c * B_LOC : (c + 1) * B_LOC]
                nc.gpsimd.affine_select(
                    m, ones_pb[:], pattern=[[-P, B_LOC]],
                    compare_op=mybir.AluOpType.is_ge, fill=0.0,
                    base=128 * c, channel_multiplier=1,
                )
                # upper bound via is_ge with negated iota (is_le not in codegen)
                nc.gpsimd.affine_select(
                    m, m, pattern=[[P, B_LOC]],
                    compare_op=mybir.AluOpType.is_ge, fill=0.0,
                    base=(P - 1) - 128 * c, channel_multiplier=-1,
                )

            # prologue: decT, then biasT = (dec @ W_dec + b_dec + b_enc).T  [a, b]
            with tc.tile_pool(name="pro_ps", bufs=2, space="PSUM") as pro_ps:
                for j in range(WJ):
                    ps_dt = pro_ps.tile([128, B_LOC], F32, name="ps_dt")
                    nc.tensor.transpose(
                        ps_dt[:], dec_sb[0:B_LOC, j * 128 : (j + 1) * 128],
                        identity[0:B_LOC, 0:B_LOC],
                    )
                    nc.vector.tensor_copy(decT_bf[:, j * B_LOC : (j + 1) * B_LOC], ps_dt[:])
                for aj in range(AJ):
                    ps_b = pro_ps.tile([128, B_LOC], F32, name="ps_b")
                    for wj in range(WJ):
                        nc.tensor.matmul(
                            ps_b[:],
                            lhsT=w_dec_sb[:, wj * A + aj * 128 : wj * A + (aj + 1) * 128],
                            rhs=decT_bf[:, wj * B_LOC : (wj + 1) * B_LOC],
                            start=(wj == 0), stop=False,
                        )
                    # rank-1 add of (b_enc + b_dec) broadcast over batch columns
                    nc.tensor.matmul(
                        ps_b[:],
                        lhsT=bb_bf[0:1, aj * 128 : (aj + 1) * 128],
                        rhs=ones32[0:1, :],
                        start=False, stop=True,
                    )
                    nc.scalar.copy(biasT_sb[:, aj * B_LOC : (aj + 1) * B_LOC], ps_b[:])

            with (
                tc.tile_pool(name="nat_pool", bufs=6) as nat_pool,
                tc.tile_pool(name="encT_pool", bufs=4) as encT_pool,
                tc.tile_pool(name="hidT_pool", bufs=4) as hidT_pool,
                tc.tile_pool(name="w6_pool", bufs=6) as w6_pool,
                tc.tile_pool(name="mm_ps", bufs=2, space="PSUM") as mm_ps,
                tc.tile_pool(name="at_ps_pool", bufs=1, space="PSUM") as at_ps_pool,
                tc.tile_pool(name="acc_ps", bufs=1, space="PSUM") as acc_ps,
            ):
                # step6 uses PE column-packing: chunk c accumulates into
                # partition strip 32*(c%4) of full-height PSUM tensors, so 4
                # chunks' matmuls run concurrently in different col-groups of
                # the array (4 XBUS streams); strips are summed at the end.
                out_ps = [
                    acc_ps.tile([128, 512], F32, name=f"out_ps{eg}") for eg in range(EG)
                ]
                z_ps = acc_ps.tile([128, 1], F32)

                nat = [None] * NCHUNK  # per-chunk [128, E] views into pair tiles
                next6 = 0
                # small leading groups so PE starts while the DMA pipe ramps
                sizes = [1, 1, 2] + [4] * ((NCHUNK - 4) // 4)
                sizes += [NCHUNK - sum(sizes)] if sum(sizes) < NCHUNK else []
                assert sum(sizes) == NCHUNK
                starts = [sum(sizes[:i]) for i in range(len(sizes))]
                for g, (cstart, nch) in enumerate(zip(starts, sizes)):
                    gr = nch * 128
                    encT = encT_pool.tile([128, EJ * 512], BF16, name="encT")
                    for pc in range(0, nch, 2):
                        c0 = cstart + pc
                        npair = min(2, nch - pc)
                        nat_t = nat_pool.tile([128, 2 * E], BF16, name="nat")
                        for i in range(npair):
                            nat[c0 + i] = nat_t[:, i * E : (i + 1) * E]
                        # one cast-DMA per chunk pair (f32 HBM -> bf16 SBUF)
                        src = enc_x[c0 * 128 : (c0 + npair) * 128, :].rearrange(
                            "(i p) e -> p i e", p=128, i=npair
                        )
                        dst = nat_t.rearrange("p (i e) -> p i e", i=2)[:, 0:npair, :]
                        nc.gpsimd.dma_start(dst, src)
                        for i in range(npair):
                            rc = pc + i
                            # whole-chunk transpose: out[p, j, r] = nat[r, 128j+p]
                            # (xbar: out = in.reshape(reversed(out.shape)).T).
                            # All transposes stay on ONE HWDGE ring: concurrent
                            # transposes on both rings corrupt data (shared xbar).
                            encT_3d = encT.rearrange("p (j r) -> p j r", j=EJ)
                            nc.sync.dma_start(
                                encT_3d[:, :, rc * 128 : rc * 128 + 128],
                                nat[c0 + i],
                                transpose=True,
                            )
                    if g == 0:
                        # second half of W_enc: into the Pool ring after group
                        # 0's loads but before any consumer in program order
                        nc.gpsimd.dma_start(
                            w_enc_sb[:, EJ // 2 * A : EJ * A],
                            wenc_x[EJ // 2 * 128 : EJ * 128, :].rearrange(
                                "(j p) a -> p j a", p=128
                            ),
                        )

                    hidT = hidT_pool.tile([128, AJ * 512], BF16, name="hidT")
                    for aj in range(AJ):
                        ps_h = mm_ps.tile([128, 512], F32, name="ps_h")
                        for ej in range(EJ):
                            nc.tensor.matmul(
                                ps_h[:, 0:gr],
                                lhsT=w_enc_sb[:, ej * A + aj * 128 : ej * A + (aj + 1) * 128],
                                rhs=encT[:, ej * 512 : ej * 512 + gr],
                                start=(ej == 0), stop=(ej == EJ - 1),
                            )
                        for b, s0, s1 in _batch_segments(128 * cstart, gr):
                            nc.scalar.activation(
                                hidT[:, aj * 512 + s0 : aj * 512 + s1],
                                ps_h[:, s0:s1],
                                AF.Relu,
                                bias=biasT_sb[:, aj * B_LOC + b : aj * B_LOC + b + 1],
                            )

                    for rc in range(nch):
                        c = cstart + rc
                        at_ps = at_ps_pool.tile([128, 1], F32, name="at_ps")
                        for aj in range(AJ):
                            nc.tensor.matmul(
                                at_ps[:],
                                lhsT=hidT[:, aj * 512 + rc * 128 : aj * 512 + rc * 128 + 128],
                                rhs=wfin_sb[:, aj : aj + 1],
                                start=(aj == 0), stop=(aj == AJ - 1),
                            )
                        # softmax numerator (no max-subtraction; |att| < ~2)
                        nc.scalar.activation(attT_sb[:, c : c + 1], at_ps[:], AF.Exp)

                    # issue weighted-sum for every chunk whose batches are all done
                    rows_done = 128 * (cstart + nch)
                    while next6 < NCHUNK:
                        last_b = (128 * next6 + 127) // P
                        if (last_b + 1) * P > rows_done:
                            break
                        c = next6
                        issue_mask(c)
                        w6 = w6_pool.tile([128, B_LOC], BF16, name="w6")
                        nc.vector.memset(w6[:], 0.0)
                        nc.vector.copy_predicated(
                            w6[:],
                            masks_sb[:, c * B_LOC : (c + 1) * B_LOC],
                            attT_sb[:, c : c + 1].broadcast_to([128, B_LOC]),
                        )
                        sj = (c % 4) * B_LOC
                        for eg in range(EG):
                            nc.tensor.matmul(
                                out_ps[eg][sj : sj + B_LOC, :],
                                lhsT=w6[:],
                                rhs=nat[c][:, eg * 512 : (eg + 1) * 512],
                                start=(c < 4), stop=(c >= NCHUNK - 4),
                                tile_position=(0, sj),
                            )
                        nc.tensor.matmul(
                            z_ps[sj : sj + B_LOC, :], lhsT=w6[:], rhs=onescol[:],
                            start=(c < 4), stop=(c >= NCHUNK - 4),
                            tile_position=(0, sj),
                        )
                        next6 += 1

                assert next6 == NCHUNK
                # cross-strip reduction: out[b] = sum_j strip[32j + b]
                red_sb = consts.tile([128, EG * 512 + 1], F32, name="red_sb")
                for eg in range(EG):
                    nc.scalar.copy(red_sb[:, eg * 512 : (eg + 1) * 512], out_ps[eg][:])
                nc.vector.tensor_copy(red_sb[:, EG * 512 : EG * 512 + 1], z_ps[:])
                zf_ps = mm_ps.tile([B_LOC, 1], F32, name="ps_h")
                nc.tensor.matmul(
                    zf_ps[:], lhsT=id4[:], rhs=red_sb[:, EG * 512 : EG * 512 + 1],
                    start=True, stop=True,
                )
                nc.vector.reciprocal(recip_z[:], zf_ps[:])
                for eg in range(EG):
                    of_ps = mm_ps.tile([B_LOC, 512], F32, name="ps_h")
                    nc.tensor.matmul(
                        of_ps[:], lhsT=id4[:],
                        rhs=red_sb[:, eg * 512 : (eg + 1) * 512],
                        start=True, stop=True,
                    )
                    nc.scalar.activation(
                        out_sb[:, eg * 512 : (eg + 1) * 512],
                        of_ps[:],
                        AF.Copy,
                        scale=recip_z[:],
                    )
                nc.sync.dma_start(out_x[:], out_sb[:])

    nc.compile()
    return nc


_NC = None


def _get_nc():
    global _NC
    if _NC is None:
        _NC = build()
    return _NC


def _in_maps(inputs):
    enc = np.ascontiguousarray(np.asarray(inputs["encoder_out"], dtype=np.float32))
    dec = np.ascontiguousarray(np.asarray(inputs["decoder_out"], dtype=np.float32))
    wenc = np.ascontiguousarray(np.asarray(inputs["W_enc"], dtype=np.float32))
    benc = np.asarray(inputs["b_enc"], dtype=np.float32).reshape(1, A)
    wdec = np.ascontiguousarray(np.asarray(inputs["W_dec"], dtype=np.float32))
    bdec = np.asarray(inputs["b_dec"], dtype=np.float32).reshape(1, A)
    wfin = np.ascontiguousarray(np.asarray(inputs["W_fin"], dtype=np.float32))

    maps = []
    for i in range(N_CORES):
        maps.append(
            {
                "encoder_out": np.ascontiguousarray(
                    enc[i * B_LOC : (i + 1) * B_LOC].reshape(ROWS, E)
                ),
                "decoder_out": np.ascontiguousarray(dec[i * B_LOC : (i + 1) * B_LOC]),
                "W_enc": wenc,
                "b_enc": benc,
                "W_dec": wdec,
                "b_dec": bdec,
                "W_fin": wfin,
            }
        )
    return maps


def run(inputs, trace=False):
    """Run the kernel; returns (out [256, 2048] f32, exec_time_ns or None)."""
    nc = _get_nc()
    res = run_bass_kernel_spmd(
        nc, _in_maps(inputs), core_ids=list(range(N_CORES)), trace=trace
    )
    out = np.concatenate([res.results[i]["out"] for i in range(N_CORES)], axis=0)
    return out.astype(np.float32), res.exec_time_ns


def kernel(**inputs):
    out, _ = run(inputs, trace=False)
    return out

